# revision 1
# baseline (speedup 1.0000x reference)
"""Trainium2 Bass kernel for a single transformer decoder layer.

Sharding: 8 cores = 4 batches x 2 head-groups (tensor parallel over heads for
attention; pairwise ReduceScatter; token-split FFN). All activations are kept
feature-major ("transposed") on device so no on-device transposes are needed;
the host transposes inputs/outputs instead. Token ownership after the
reduce-scatter is by interleaved 512-blocks ({0,2} / {1,3}) so the first
collective can fire early and overlap the tail of attention.
"""

import sys

for _p in ("/opt/trn_rl_repo",):
    if _p not in sys.path:
        sys.path.insert(0, _p)

import numpy as np

import concourse.bass as bass
import concourse.mybir as mybir
import concourse.tile as tile
from concourse import bacc
from concourse.bass_utils import run_bass_kernel_spmd

# ---- problem constants (hardcoded per spec) ----
B, S, D = 4, 2048, 1024
H, DK, DV, DFF = 16, 64, 64, 4096
EPS = 1e-5
SCALE = 1.0 / 32.0  # 1/sqrt(D)

NCORES = 8
HL = H // 2          # heads per core (local)
NP = HL // 2         # head-pairs per core (4)
TLOC = S // 2        # tokens owned per core after reduce-scatter (1024)
DC = D // 128        # d-model chunks (8)
FC = DFF // 128      # dff chunks (32)
FQ = FC // 4         # dff chunks per quarter (8)
QB = S // 512        # query blocks of 512 (4)

F32 = mybir.dt.float32
F32R = mybir.dt.float32r
BF16 = mybir.dt.bfloat16

DEBUG = False
_COMPILED = None


def _build():
    nc = bacc.Bacc("TRN2", target_bir_lowering=False, debug=False,
                   num_devices=NCORES)

    xT_d = nc.dram_tensor("xT", [D, S], F32, kind="ExternalInput").ap()
    xTm_d = nc.dram_tensor("xTmine", [D, TLOC], F32, kind="ExternalInput").ap()
    wq_d = nc.dram_tensor("wq", [NP, 128, DC, 128], F32, kind="ExternalInput").ap()
    wk_d = nc.dram_tensor("wk", [NP, 128, DC, 128], F32, kind="ExternalInput").ap()
    wv_d = nc.dram_tensor("wv", [128, DC, 512], F32, kind="ExternalInput").ap()
    wo_d = nc.dram_tensor("wo", [128, NP, DC, 128], F32, kind="ExternalInput").ap()
    w1_d = nc.dram_tensor("w1", [FC, 128, DC, 128], BF16, kind="ExternalInput").ap()
    w2_d = nc.dram_tensor("w2", [FC, 128, DC, 128], BF16, kind="ExternalInput").ap()
    b1_d = nc.dram_tensor("b1s", [128, FC], F32, kind="ExternalInput").ap()
    b2_d = nc.dram_tensor("b2s", [128, DC], F32, kind="ExternalInput").ap()
    g1_d = nc.dram_tensor("g1s", [128, DC], F32, kind="ExternalInput").ap()
    e1_d = nc.dram_tensor("e1s", [128, DC], F32, kind="ExternalInput").ap()
    g2_d = nc.dram_tensor("g2s", [128, DC], F32, kind="ExternalInput").ap()
    e2_d = nc.dram_tensor("e2s", [128, DC], F32, kind="ExternalInput").ap()
    mk_d = nc.dram_tensor("mask", [128, 4, 512], F32, kind="ExternalInput").ap()

    outT_d = nc.dram_tensor("outT", [D, TLOC], F32, kind="ExternalOutput").ap()
    dbg = None
    if DEBUG:
        dbg = {
            "dbg_ctx": nc.dram_tensor("dbg_ctx", [128, NP, S], F32,
                                      kind="ExternalOutput").ap(),
            "dbg_rso": nc.dram_tensor("dbg_rso", [D, TLOC], F32,
                                      kind="ExternalOutput").ap(),
            "dbg_h1p": nc.dram_tensor("dbg_h1p", [128, DC, TLOC], F32,
                                      kind="ExternalOutput").ap(),
            "dbg_h1": nc.dram_tensor("dbg_h1", [128, DC, TLOC], F32,
                                     kind="ExternalOutput").ap(),
            "dbg_o2": nc.dram_tensor("dbg_o2", [128, DC, TLOC], F32,
                                     kind="ExternalOutput").ap(),
        }

    with tile.TileContext(nc) as tc:
        _emit(nc, tc, xT_d, xTm_d, wq_d, wk_d, wv_d, wo_d, w1_d, w2_d,
              b1_d, b2_d, g1_d, e1_d, g2_d, e2_d, mk_d, outT_d, dbg)
    nc.compile()
    return nc


def _emit(nc, tc, xT_d, xTm_d, wq_d, wk_d, wv_d, wo_d, w1_d, w2_d,
          b1_d, b2_d, g1_d, e1_d, g2_d, e2_d, mk_d, outT_d, dbg=None):
    AF = mybir.ActivationFunctionType

    with (
        tc.tile_pool(name="dram", bufs=1, space="DRAM") as dram,
        tc.tile_pool(name="const", bufs=1) as const,
    ):
        MASK = const.tile([128, 4, 512], F32R)
        nc.gpsimd.dma_start(MASK[:], mk_d[:])
        onesf = const.tile([128, 1], F32)
        nc.vector.memset(onesf[:], 1.0)
        ones1 = const.tile([128, 1], F32R)
        nc.vector.tensor_copy(ones1[:], onesf[:])
        epst = const.tile([1, 1], F32)
        nc.vector.memset(epst[:], EPS)
        g1t = const.tile([128, DC], F32)
        e1t = const.tile([128, DC], F32)
        g2t = const.tile([128, DC], F32)
        e2t = const.tile([128, DC], F32)
        b1t = const.tile([128, FC], F32)
        b2t = const.tile([128, DC], F32)
        for t_, d_ in ((g1t, g1_d), (e1t, e1_d), (g2t, g2_d), (e2t, e2_d),
                       (b1t, b1_d), (b2t, b2_d)):
            nc.sync.dma_start(t_[:], d_[:])

        rs_in0 = dram.tile([2, D, 512], F32)
        rs_in1 = dram.tile([2, D, 512], F32)
        rs_out0 = dram.tile([D, 512], F32)
        rs_out1 = dram.tile([D, 512], F32)

        def layer_norm(src, dst, gt, et, work, psStat, statp):
            """feature-major LN over features of a [128, DC, 512] block."""
            sq = work.tile([128, DC, 512], F32R, tag="sq")
            nc.scalar.activation(sq[:], src, AF.Square)
            pmu = psStat.tile([1, 512], F32, tag="stat")
            psq = psStat.tile([1, 512], F32, tag="stat")
            for dc in range(DC):
                nc.tensor.matmul(pmu[:], ones1[:], src[:, dc],
                                 start=(dc == 0), stop=(dc == DC - 1))
            for dc in range(DC):
                nc.tensor.matmul(psq[:], ones1[:], sq[:, dc],
                                 start=(dc == 0), stop=(dc == DC - 1))
            mu = statp.tile([1, 512], F32, tag="mu")
            ex2 = statp.tile([1, 512], F32, tag="ex2")
            nc.vector.tensor_scalar_mul(mu[:], pmu[:], 1.0 / D)
            nc.vector.tensor_scalar_mul(ex2[:], psq[:], 1.0 / D)
            var = statp.tile([1, 512], F32, tag="var")
            nc.vector.tensor_mul(var[:], mu[:], mu[:])
            nc.vector.tensor_sub(var[:], ex2[:], var[:])
            srt = statp.tile([1, 512], F32, tag="srt")
            nc.scalar.activation(srt[:], var[:], AF.Sqrt, bias=epst[:])
            rstd = statp.tile([1, 512], F32, tag="rstd")
            nc.vector.reciprocal(rstd[:], srt[:])
            nmr = statp.tile([1, 512], F32, tag="nmr")
            nc.vector.tensor_mul(nmr[:], mu[:], rstd[:])
            nc.vector.tensor_scalar_mul(nmr[:], nmr[:], -1.0)
            rstdb = work.tile([128, 512], F32, tag="bc1")
            nmrb = work.tile([128, 512], F32, tag="bc2")
            nc.gpsimd.partition_broadcast(rstdb[:], rstd[:])
            nc.gpsimd.partition_broadcast(nmrb[:], nmr[:])
            xh = work.tile([128, DC, 512], F32, tag="sq")
            nc.vector.tensor_mul(xh[:], src,
                                 rstdb[:, None, :].to_broadcast((128, DC, 512)))
            nc.vector.tensor_add(xh[:], xh[:],
                                 nmrb[:, None, :].to_broadcast((128, DC, 512)))
            for dc in range(DC):
                nc.scalar.activation(dst[:, dc], xh[:, dc], AF.Identity,
                                     bias=et[:, dc:dc + 1],
                                     scale=gt[:, dc:dc + 1])

        # ============ attention: projections + qb-major attention ============
        with (
            tc.tile_pool(name="pQKT", bufs=1) as pQKT,
            tc.tile_pool(name="pV", bufs=1) as pV,
        ):
            QT = pQKT.tile([128, NP, S], F32R, tag="QT")          # 32KB
            KT = pQKT.tile([128, NP, S], F32R, tag="KT")          # 32KB
            V = pV.tile([128, S // 128, HL * 65], F32R, tag="V")  # 33.3KB
            nc.vector.tensor_copy(
                V[:].rearrange("p t (h c) -> p t h c", c=65)[:, :, :, 64:65],
                onesf[:, None, None, :].to_broadcast((128, S // 128, HL, 1)))

            with (
                tc.tile_pool(name="pX", bufs=1) as pX,
                tc.tile_pool(name="pXs", bufs=2) as pXs,
                tc.tile_pool(name="pWQK", bufs=1) as pWQK,
            ):
                X = pX.tile([128, DC, S], F32R, tag="X")          # 64KB
                for dc in range(DC):
                    for hh in range(2):
                        xs = pXs.tile([128, 1024], F32, tag="xs")
                        nc.sync.dma_start(
                            xs[:],
                            xT_d.rearrange("(dc p) t -> p dc t",
                                           p=128)[:, dc,
                                                  hh * 1024:(hh + 1) * 1024])
                        nc.vector.tensor_copy(
                            X[:, dc, hh * 1024:(hh + 1) * 1024], xs[:])

                # Q/K projections, dc-outer so PE starts on the first X chunk
                with tc.tile_pool(name="psP", bufs=8, space="PSUM") as psP:
                    for p in range(NP):
                        wqt = pWQK.tile([128, DC, 128], F32R, tag="wq")
                        wkt = pWQK.tile([128, DC, 128], F32R, tag="wk")
                        nc.gpsimd.dma_start(wqt[:], wq_d[p])
                        nc.gpsimd.dma_start(wkt[:], wk_d[p])
                        pqs = [psP.tile([128, 512], F32, tag="proj",
                                        name=f"pq_{i}") for i in range(8)]
                        for dc in range(DC):
                            for tb in range(QB):
                                nc.tensor.matmul(
                                    pqs[tb][:], wqt[:, dc],
                                    X[:, dc, tb * 512:(tb + 1) * 512],
                                    start=(dc == 0), stop=(dc == DC - 1))
                            for tb in range(QB):
                                nc.tensor.matmul(
                                    pqs[4 + tb][:], wkt[:, dc],
                                    X[:, dc, tb * 512:(tb + 1) * 512],
                                    start=(dc == 0), stop=(dc == DC - 1))
                        for tb in range(QB):
                            tsl = slice(tb * 512, (tb + 1) * 512)
                            nc.vector.tensor_scalar_mul(QT[:, p, tsl],
                                                        pqs[tb][:], SCALE)
                            nc.vector.tensor_copy(KT[:, p, tsl], pqs[4 + tb][:])

                # V projection (needs all of X)
                with (
                    tc.tile_pool(name="psV", bufs=3, space="PSUM") as psV,
                    tc.tile_pool(name="pWV", bufs=1) as pWV,
                ):
                    wvt = pWV.tile([128, DC, 512], F32R, tag="wv")
                    nc.gpsimd.dma_start(wvt[:], wv_d[:])
                    for tt in range(S // 128):
                        pv = psV.tile([128, 512], F32, tag="pv")
                        for dc in range(DC):
                            nc.tensor.matmul(pv[:],
                                             X[:, dc, tt * 128:(tt + 1) * 128],
                                             wvt[:, dc],
                                             start=(dc == 0), stop=(dc == DC - 1))
                        nc.vector.tensor_copy(
                            V[:, tt].rearrange("p (h c) -> p h c",
                                               c=65)[:, :, 0:64],
                            pv[:].rearrange("p (h c) -> p h c", c=64))

            # ---- attention, qb-outer; Wo + reduce-scatter interleaved ----
            with (
                tc.tile_pool(name="pCTX", bufs=1) as pCTX,
                tc.tile_pool(name="pWO", bufs=1) as pWO,
                tc.tile_pool(name="pE", bufs=4) as pE,
                tc.tile_pool(name="pAO", bufs=3) as pAO,
                tc.tile_pool(name="stB", bufs=2) as stB,
                tc.tile_pool(name="psS", bufs=2, space="PSUM") as psS,
                tc.tile_pool(name="psC", bufs=4, space="PSUM") as psC,
                tc.tile_pool(name="psW", bufs=2, space="PSUM") as psW,
            ):
                CTX = pCTX.tile([128, NP, S], F32R, tag="CTX")    # 32KB
                wot = pWO.tile([128, NP, DC, 128], F32R, tag="wo")
                nc.gpsimd.dma_start(wot[:], wo_d[:])

                for qb in range(QB):
                    qsl = slice(qb * 512, (qb + 1) * 512)
                    nkc = 4 * (qb + 1)
                    for p in range(NP):
                        ctxA = psC.tile([65, 512], F32, tag="ctx")
                        ctxB = psC.tile([65, 512], F32, tag="ctx")
                        for kc in range(nkc):
                            ksl = slice(kc * 128, (kc + 1) * 128)
                            sA = psS.tile([128, 512], F32, tag="sc")
                            sB = psS.tile([128, 512], F32, tag="sc")
                            nc.tensor.matmul(sA[:], KT[0:64, p, ksl],
                                             QT[0:64, p, qsl],
                                             start=True, stop=True)
                            nc.tensor.matmul(sB[:], KT[64:128, p, ksl],
                                             QT[64:128, p, qsl],
                                             start=True, stop=True)
                            eA = pE.tile([128, 512], F32R, tag="E")
                            eB = pE.tile([128, 512], F32R, tag="E")
                            nc.scalar.activation(eA[:], sA[:], AF.Exp)
                            nc.scalar.activation(eB[:], sB[:], AF.Exp)
                            if kc >= 4 * qb:
                                mkc = kc - 4 * qb
                                nc.vector.tensor_mul(eA[:], eA[:], MASK[:, mkc])
                                nc.vector.tensor_mul(eB[:], eB[:], MASK[:, mkc])
                            st, sp = (kc == 0), (kc == nkc - 1)
                            nc.tensor.matmul(
                                ctxA[:], V[:, kc, 2 * p * 65:(2 * p + 1) * 65],
                                eA[:], start=st, stop=sp)
                            nc.tensor.matmul(
                                ctxB[:],
                                V[:, kc, (2 * p + 1) * 65:(2 * p + 2) * 65],
                                eB[:], start=st, stop=sp)
                        for row0, cxt in ((0, ctxA), (64, ctxB)):
                            rec = stB.tile([1, 512], F32, tag="rec")
                            nc.vector.reciprocal(rec[:], cxt[64:65, :])
                            recb = stB.tile([64, 512], F32, tag="recb")
                            nc.gpsimd.partition_broadcast(recb[:], rec[:])
                            nc.vector.tensor_mul(CTX[row0:row0 + 64, p, qsl],
                                                 cxt[0:64, :], recb[:])
                    # Wo partial for this token block
                    rsdst = rs_in0 if qb < 2 else rs_in1
                    for dout in range(DC):
                        po = psW.tile([128, 512], F32, tag="wo")
                        for p in range(NP):
                            nc.tensor.matmul(po[:], wot[:, p, dout],
                                             CTX[:, p, qsl],
                                             start=(p == 0), stop=(p == NP - 1))
                        ao = pAO.tile([128, 512], F32, tag="ao")
                        nc.vector.tensor_copy(ao[:], po[:])
                        nc.sync.dma_start(
                            rsdst[qb % 2, dout * 128:(dout + 1) * 128, :],
                            ao[:])
                    if qb == 1:
                        nc.gpsimd.collective_compute(
                            "ReduceScatter", mybir.AluOpType.add,
                            replica_groups=[[0, 1], [2, 3], [4, 5], [6, 7]],
                            ins=[rs_in0.opt()], outs=[rs_out0.opt()])
                if dbg is not None:
                    nc.gpsimd.dma_start(dbg["dbg_ctx"][:], CTX[:])


        # ======== D-half0 before the second collective ========
        with (
            tc.tile_pool(name="pW1q", bufs=1) as pW1q,
            tc.tile_pool(name="pH1", bufs=1) as pH1,
            tc.tile_pool(name="pH1b", bufs=1) as pH1b,
            tc.tile_pool(name="pAOr", bufs=1) as pAOr,
            tc.tile_pool(name="pLN", bufs=1) as pLN,
            tc.tile_pool(name="stDE", bufs=1) as stDE,
            tc.tile_pool(name="psD", bufs=2, space="PSUM") as psD,
        ):
            H1h = [pH1.tile([128, DC, 512], F32R, tag=f"H1_{h}",
                            name=f"H1_{h}") for h in range(2)]
            H1b = [pH1b.tile([128, DC, 512], BF16, tag=f"H1b{h}",
                             name=f"H1b{h}") for h in range(2)]

            def d_half(h, rso):
                aor = pAOr.tile([128, DC, 512], F32R, tag="AOr",
                                name=f"AOr{h}")
                nc.gpsimd.dma_start(
                    aor[:], rso.rearrange("(dc p) t -> p dc t", p=128))
                xm = pAOr.tile([128, DC, 512], F32, tag="XM",
                               name=f"XMt{h}")
                nc.sync.dma_start(
                    xm[:], xTm_d.rearrange("(dc p) t -> p dc t",
                                           p=128)[:, :, h * 512:(h + 1) * 512])
                nc.vector.tensor_add(aor[:], aor[:], xm[:])
                if dbg is not None:
                    nc.gpsimd.dma_start(
                        dbg["dbg_rso"][:, h * 512:(h + 1) * 512], rso[:])
                    nc.gpsimd.dma_start(
                        dbg["dbg_h1p"][:, :, h * 512:(h + 1) * 512], aor[:])
                layer_norm(aor[:], H1h[h][:], g1t, e1t, pLN, psD, stDE)
                nc.vector.tensor_copy(H1b[h][:], H1h[h][:])
                if dbg is not None:
                    nc.gpsimd.dma_start(
                        dbg["dbg_h1"][:, :, h * 512:(h + 1) * 512],
                        H1h[h][:])

            d_half(0, rs_out0)
            # second reduce-scatter (gpsimd blocks on collectives, so all
            # pre-RS-B gpsimd work is already queued above)
            nc.gpsimd.collective_compute(
                "ReduceScatter", mybir.AluOpType.add,
                replica_groups=[[0, 1], [2, 3], [4, 5], [6, 7]],
                ins=[rs_in1.opt()], outs=[rs_out1.opt()])

            # ======== FFN th-major (bf16) + residual + LN2 ========
            with (
                tc.tile_pool(name="pFF", bufs=1) as pFF,
                tc.tile_pool(name="pO2", bufs=1) as pO2,
                tc.tile_pool(name="pW2q", bufs=2) as pW2q,
                tc.tile_pool(name="psF", bufs=2, space="PSUM") as psF,
                tc.tile_pool(name="psG", bufs=4, space="PSUM") as psG,
            ):
                FFt = pFF.tile([128, FQ, 512], BF16, tag="FF")  # 8KB
                O2h = [pO2.tile([128, DC, 512], F32R, tag=f"O2_{h}",
                                name=f"O2_{h}") for h in range(2)]

                def ffn_half(th):
                    for fq in range(4):
                        w1q = pW1q.tile([128, FQ, DC, 128], BF16,
                                        tag="w1", name=f"w1q_{th}_{fq}")
                        nc.sync.dma_start(
                            w1q[:],
                            w1_d[fq * FQ:(fq + 1) * FQ].rearrange(
                                "f p dc n -> p f dc n"))
                        for fi in range(FQ):
                            fc = fq * FQ + fi
                            pf = psF.tile([128, 512], F32, tag="ff")
                            for dc in range(DC):
                                nc.tensor.matmul(
                                    pf[:], w1q[:, fi, dc], H1b[th][:, dc],
                                    start=(dc == 0), stop=(dc == DC - 1))
                            nc.scalar.activation(FFt[:, fi], pf[:],
                                                 AF.Relu,
                                                 bias=b1t[:, fc:fc + 1])
                        for dq in range(4):
                            w2q = pW2q.tile([128, FQ, 2, 128], BF16,
                                            tag="w2")
                            nc.sync.dma_start(
                                w2q[:],
                                w2_d[fq * FQ:(fq + 1) * FQ, :,
                                     dq * 2:(dq + 1) * 2].rearrange(
                                         "f p d n -> p f d n"))
                            pos = [psG.tile([128, 512], F32, tag="o2",
                                            name=f"o2_{i}")
                                   for i in range(2)]
                            for fi in range(FQ):
                                for do2 in range(2):
                                    nc.tensor.matmul(
                                        pos[do2][:], w2q[:, fi, do2],
                                        FFt[:, fi],
                                        start=(fi == 0), stop=(fi == FQ - 1))
                            for do2 in range(2):
                                dout = dq * 2 + do2
                                if fq == 0:
                                    nc.vector.tensor_copy(
                                        O2h[th][:, dout], pos[do2][:])
                                else:
                                    nc.vector.tensor_add(
                                        O2h[th][:, dout],
                                        O2h[th][:, dout], pos[do2][:])

                def finish_half(th, psL):
                    nc.vector.tensor_add(O2h[th][:], O2h[th][:],
                                         H1h[th][:])
                    for dc in range(DC):
                        nc.vector.tensor_scalar_add(O2h[th][:, dc],
                                                    O2h[th][:, dc],
                                                    b2t[:, dc:dc + 1])
                    if dbg is not None:
                        nc.gpsimd.dma_start(
                            dbg["dbg_o2"][:, :, th * 512:(th + 1) * 512],
                            O2h[th][:])
                    ot = pFF.tile([128, DC, 512], F32, tag="ot",
                                  name=f"ot{th}")
                    layer_norm(O2h[th][:], ot, g2t, e2t, pLN, psL, stDE)
                    nc.sync.dma_start(
                        outT_d.rearrange(
                            "(dc p) t -> p dc t",
                            p=128)[:, :, th * 512:(th + 1) * 512],
                        ot[:])

                ffn_half(0)
                d_half(1, rs_out1)
                finish_half(0, psD)
                ffn_half(1)
                finish_half(1, psD)


def _pack_inputs(x, Wq, Wk, Wv, Wo, ln1_g, ln1_b, W1, b1, W2, b2, ln2_g, ln2_b):
    """Build the 8 per-core input maps (all host-side numpy)."""
    f = np.float32
    x = np.asarray(x, f)
    Wq = np.asarray(Wq, f); Wk = np.asarray(Wk, f); Wv = np.asarray(Wv, f)
    Wo = np.asarray(Wo, f); W1 = np.asarray(W1, f); W2 = np.asarray(W2, f)
    in_maps = []
    import ml_dtypes
    w1p = np.ascontiguousarray(
        W1.reshape(DC, 128, FC, 128).transpose(2, 1, 0, 3)).astype(
            ml_dtypes.bfloat16)
    w2p = np.ascontiguousarray(W2.reshape(FC, 128, DC, 128)).astype(
        ml_dtypes.bfloat16)
    b1s = np.ascontiguousarray(np.asarray(b1, f).reshape(FC, 128).T)
    b2s = np.ascontiguousarray(np.asarray(b2, f).reshape(DC, 128).T)
    g1s = np.ascontiguousarray(np.asarray(ln1_g, f).reshape(DC, 128).T)
    e1s = np.ascontiguousarray(np.asarray(ln1_b, f).reshape(DC, 128).T)
    g2s = np.ascontiguousarray(np.asarray(ln2_g, f).reshape(DC, 128).T)
    e2s = np.ascontiguousarray(np.asarray(ln2_b, f).reshape(DC, 128).T)
    kk = np.arange(512)[:, None]
    qq = np.arange(512)[None, :]
    mask = (kk <= qq).astype(f).reshape(4, 128, 512).transpose(1, 0, 2)
    mask = np.ascontiguousarray(mask)

    for c in range(NCORES):
        b, j = c // 2, c % 2
        hb = j * HL
        xT = np.ascontiguousarray(x[b].T)
        # owned token blocks: {j, j+2} of four 512-blocks
        xTm = np.ascontiguousarray(np.concatenate(
            [x[b, j * 512:(j + 1) * 512],
             x[b, (j + 2) * 512:(j + 3) * 512]]).T)
        wq = np.stack([np.concatenate([Wq[hb + 2 * p], Wq[hb + 2 * p + 1]], 1)
                       for p in range(NP)])  # [NP, D, 128]
        wq = np.ascontiguousarray(
            wq.reshape(NP, DC, 128, 128).transpose(0, 2, 1, 3))
        wk = np.stack([np.concatenate([Wk[hb + 2 * p], Wk[hb + 2 * p + 1]], 1)
                       for p in range(NP)])
        wk = np.ascontiguousarray(
            wk.reshape(NP, DC, 128, 128).transpose(0, 2, 1, 3))
        wv = np.concatenate([Wv[hb + i] for i in range(HL)], 1)  # [D, 512]
        wv = np.ascontiguousarray(
            wv.reshape(DC, 128, 512).transpose(1, 0, 2))
        wo = Wo[j * 512:(j + 1) * 512]  # [512, D]
        wo = np.ascontiguousarray(
            wo.reshape(NP, 128, DC, 128).transpose(1, 0, 2, 3))
        in_maps.append({
            "xT": xT, "xTmine": xTm, "wq": wq, "wk": wk, "wv": wv, "wo": wo,
            "w1": w1p, "w2": w2p, "b1s": b1s, "b2s": b2s,
            "g1s": g1s, "e1s": e1s, "g2s": g2s, "e2s": e2s, "mask": mask,
        })
    return in_maps


def get_compiled():
    global _COMPILED
    if _COMPILED is None:
        _COMPILED = _build()
    return _COMPILED


def kernel(x, Wq, Wk, Wv, Wo, ln1_g, ln1_b, W1, b1, W2, b2, ln2_g, ln2_b,
           _trace=False):
    nc = get_compiled()
    in_maps = _pack_inputs(x, Wq, Wk, Wv, Wo, ln1_g, ln1_b, W1, b1, W2, b2,
                           ln2_g, ln2_b)
    res = run_bass_kernel_spmd(nc, in_maps, core_ids=list(range(NCORES)),
                               trace=_trace)
    out = np.zeros((B, S, D), np.float32)
    for c in range(NCORES):
        b, j = c // 2, c % 2
        o = res.results[c]["outT"]  # [D, TLOC]; cols = blocks {j, j+2}
        out[b, j * 512:(j + 1) * 512, :] = o[:, 0:512].T
        out[b, (j + 2) * 512:(j + 3) * 512, :] = o[:, 512:1024].T
    kernel.last_result = res
    return out



# revision 10
# speedup vs baseline: 1.1933x; 1.1933x over previous
"""Trainium2 Bass kernel for a single transformer decoder layer.

Sharding: pure data-parallel over tokens, NO collectives. 8 cores =
4 batches x 2 token-groups. Core (b, j) owns two 512-token blocks of
batch b (j=0: real blocks {0,3}; j=1: {1,2} -- balanced causal work),
computes K/V for the full sequence and Q only for its own tokens (all
16 heads), runs causal attention, Wo, LN1, FFN, LN2 for its tokens,
and writes its own [D, 1024] output slice. Host reassembles.

SPMD uniformity: all cores run the same program. The asymmetric causal
structure is encoded in per-core DATA: X's token-block order is
permuted per core so q-slot A covers kc chunks 0..7 and q-slot B
covers kc 0..15, with per-core mask planes (diag / ones / zeros)
multiplied into the exp'd scores at fixed positions (slot A: kc 0..7,
slot B: kc 8..15). Each core wastes exactly 4 fully-masked kc blocks
-- the price of one shared program.

Perf: the PE p-state ramps to full clock only when continuously busy,
so everything is structured to keep an uninterrupted matmul stream:
slot-B's exp burst (scalar engine) overlaps the projection phase;
slot-A attention (ACT-bound) overlaps FFN-B's W1 matmuls; FFN W2
accumulates over all 32 ff-chunks in PSUM (no SBUF round trips).
"""

import sys

for _p in ("/opt/trn_rl_repo",):
    if _p not in sys.path:
        sys.path.insert(0, _p)

from contextlib import ExitStack

import numpy as np

import concourse.bass as bass  # noqa: F401
import concourse.mybir as mybir
import concourse.tile as tile
from concourse import bacc
from concourse.bass_utils import run_bass_kernel_spmd

# ---- problem constants (hardcoded per spec) ----
B, S, D = 4, 2048, 1024
H, DK, DV, DFF = 16, 64, 64, 4096
EPS = 1e-5
SCALE = 1.0 / 32.0  # 1/sqrt(D), folded into Wq host-side

NCORES = 8
P8 = H // 2          # head-pairs (8)
DC = D // 128        # d-model chunks (8)
FC = DFF // 128      # dff chunks (32)
TLOC = 1024          # owned tokens per core
NKC_A, NKC_B = 8, 16  # kc chunks processed for q-slot A / B (uniform)

F32 = mybir.dt.float32
F32R = mybir.dt.float32r
BF16 = mybir.dt.bfloat16
AF = mybir.ActivationFunctionType
ALU = mybir.AluOpType

DEBUG = False
_COMPILED = None


def _build():
    nc = bacc.Bacc("TRN2", target_bir_lowering=False, debug=False,
                   num_devices=NCORES)

    xT_d = nc.dram_tensor("xT", [D, S], F32, kind="ExternalInput").ap()
    xq_d = nc.dram_tensor("xq", [D, TLOC], F32, kind="ExternalInput").ap()
    wq_d = nc.dram_tensor("wq", [P8, 128, DC, 128], BF16, kind="ExternalInput").ap()
    wk_d = nc.dram_tensor("wk", [P8, 128, DC, 128], BF16, kind="ExternalInput").ap()
    wv_d = nc.dram_tensor("wv", [128, DC, 1024], BF16, kind="ExternalInput").ap()
    wo_d = nc.dram_tensor("wo", [128, P8, DC, 128], BF16, kind="ExternalInput").ap()
    w1_d = nc.dram_tensor("w1", [FC, 128, DC, 128], BF16, kind="ExternalInput").ap()
    w2_d = nc.dram_tensor("w2", [FC, 128, DC, 128], BF16, kind="ExternalInput").ap()
    mk_d = nc.dram_tensor("mask", [128, 16, 512], BF16, kind="ExternalInput").ap()
    b1_d = nc.dram_tensor("b1s", [128, FC], F32, kind="ExternalInput").ap()
    b2_d = nc.dram_tensor("b2s", [128, DC], F32, kind="ExternalInput").ap()
    g1_d = nc.dram_tensor("g1s", [128, DC], F32, kind="ExternalInput").ap()
    e1_d = nc.dram_tensor("e1s", [128, DC], F32, kind="ExternalInput").ap()
    g2_d = nc.dram_tensor("g2s", [128, DC], F32, kind="ExternalInput").ap()
    e2_d = nc.dram_tensor("e2s", [128, DC], F32, kind="ExternalInput").ap()

    outT_d = nc.dram_tensor("outT", [D, TLOC], F32, kind="ExternalOutput").ap()
    dbg = None
    if DEBUG:
        dbg = {
            "dbg_ctx": nc.dram_tensor("dbg_ctx", [128, P8, TLOC], F32,
                                      kind="ExternalOutput").ap(),
            "dbg_h1p": nc.dram_tensor("dbg_h1p", [128, DC, TLOC], F32,
                                      kind="ExternalOutput").ap(),
            "dbg_h1": nc.dram_tensor("dbg_h1", [128, DC, TLOC], F32,
                                     kind="ExternalOutput").ap(),
            "dbg_o2": nc.dram_tensor("dbg_o2", [128, DC, TLOC], F32,
                                     kind="ExternalOutput").ap(),
        }

    with tile.TileContext(nc) as tc:
        _emit(nc, tc, xT_d, xq_d, wq_d, wk_d, wv_d, wo_d, w1_d, w2_d,
              mk_d, b1_d, b2_d, g1_d, e1_d, g2_d, e2_d, outT_d, dbg)
    nc.compile()
    return nc


def _emit(nc, tc, xT_d, xq_d, wq_d, wk_d, wv_d, wo_d, w1_d, w2_d,
          mk_d, b1_d, b2_d, g1_d, e1_d, g2_d, e2_d, outT_d, dbg=None):
    esMain = ExitStack()
    with esMain:
        const = esMain.enter_context(tc.tile_pool(name="const", bufs=1))
        onesf = const.tile([128, 1], F32)
        nc.vector.memset(onesf[:], 1.0)
        ones1 = const.tile([128, 1], F32R)
        nc.vector.tensor_copy(ones1[:], onesf[:])
        onesb = const.tile([128, 1], BF16)
        nc.vector.tensor_copy(onesb[:], onesf[:])
        epst = const.tile([1, 1], F32)
        nc.vector.memset(epst[:], EPS)
        b1t = const.tile([128, FC], F32)
        b2t = const.tile([128, DC], F32)
        g1t = const.tile([128, DC], F32)
        e1t = const.tile([128, DC], F32)
        g2t = const.tile([128, DC], F32)
        e2t = const.tile([128, DC], F32)
        for t_, d_ in ((b1t, b1_d), (b2t, b2_d), (g1t, g1_d), (e1t, e1_d),
                       (g2t, g2_d), (e2t, e2_d)):
            nc.sync.dma_start(t_[:], d_[:])
        pBC = esMain.enter_context(tc.tile_pool(name="bc", bufs=2))

        # ---- attention-lifetime pools (until Wo/LN1 of slot A done) ----
        esAttn = ExitStack()
        pMA = esAttn.enter_context(tc.tile_pool(name="ma", bufs=1))
        pKTlo = esAttn.enter_context(tc.tile_pool(name="ktlo", bufs=1))
        pQTA = esAttn.enter_context(tc.tile_pool(name="qta", bufs=1))
        pV = esAttn.enter_context(tc.tile_pool(name="vlo", bufs=1))
        pCTX = esAttn.enter_context(tc.tile_pool(name="ctx", bufs=1))
        pXQ = esAttn.enter_context(tc.tile_pool(name="xq", bufs=1))
        pE = esAttn.enter_context(tc.tile_pool(name="pe", bufs=3))
        pDEN = esAttn.enter_context(tc.tile_pool(name="den", bufs=1))
        pDT = esAttn.enter_context(tc.tile_pool(name="dt", bufs=2))
        psS = esAttn.enter_context(
            tc.tile_pool(name="psS", bufs=3, space="PSUM"))
        psC = esAttn.enter_context(
            tc.tile_pool(name="psC", bufs=2, space="PSUM"))
        psW = esAttn.enter_context(
            tc.tile_pool(name="psW", bufs=1, space="PSUM"))

        MASKA = pMA.tile([128, 8, 512], BF16, tag="ma")
        nc.sync.dma_start(MASKA[:], mk_d[:, 0:8])
        KTlo = pKTlo.tile([128, P8, 1024], BF16, tag="ktlo")
        QTA = pQTA.tile([128, P8, 512], BF16, tag="qta")
        Vlo = pV.tile([128, 8, 16 * 65], BF16, tag="vlo")
        XQ = pXQ.tile([128, DC, TLOC], BF16, tag="xq")

        # ---- slot-B-lifetime pools ----
        esB = ExitStack()
        pMB = esB.enter_context(tc.tile_pool(name="mb", bufs=1))
        pKThi = esB.enter_context(tc.tile_pool(name="kthi", bufs=1))
        pQTB = esB.enter_context(tc.tile_pool(name="qtb", bufs=1))
        pVhi = esB.enter_context(tc.tile_pool(name="vhi", bufs=1))
        MASKB = pMB.tile([128, 8, 512], BF16, tag="mb")
        nc.sync.dma_start(MASKB[:], mk_d[:, 8:16])
        KThi = pKThi.tile([128, P8, 1024], BF16, tag="kthi")
        QTB = pQTB.tile([128, P8, 512], BF16, tag="qtb")
        Vhi = pVhi.tile([128, 8, 16 * 65], BF16, tag="vhi")

        for vt in (Vlo, Vhi):
            nc.vector.tensor_copy(
                vt[:].rearrange("p t (h c) -> p t h c", c=65)[:, :, :, 64:65],
                onesb[:, None, None, :].to_broadcast((128, 8, 16, 1)))

        def kt_sl(kc, rows, p):
            t = KTlo if kc < 8 else KThi
            k = kc % 8
            return t[rows, p, k * 128:(k + 1) * 128]

        def v_sl(kc, head):
            t = Vlo if kc < 8 else Vhi
            k = kc % 8
            return t[:, k, head * 65:head * 65 + 65]

        # ================= projection phase =================
        esP = ExitStack()
        pX = esP.enter_context(tc.tile_pool(name="px", bufs=1))
        pStage = esP.enter_context(tc.tile_pool(name="stg", bufs=2))
        psP = esP.enter_context(tc.tile_pool(name="psP", bufs=2, space="PSUM"))

        X = pX.tile([128, DC, S], BF16, tag="x")
        for dc in range(DC):
            for hh in range(2):
                xs = pStage.tile([128, 1024], F32, tag="xs")
                nc.sync.dma_start(
                    xs[:], xT_d.rearrange("(dc p) t -> p dc t", p=128)
                    [:, dc, hh * 1024:(hh + 1) * 1024])
                eng = nc.vector if (dc + hh) % 2 == 0 else nc.gpsimd
                eng.tensor_copy(X[:, dc, hh * 1024:(hh + 1) * 1024], xs[:])
        for dc in range(DC):
            xs = pStage.tile([128, 1024], F32, tag="xs")
            nc.sync.dma_start(
                xs[:], xq_d.rearrange("(dc p) t -> p dc t", p=128)[:, dc])
            nc.vector.tensor_copy(XQ[:, dc], xs[:])

        # K + Q projections, pair-major so attention slot B can start early
        with tc.tile_pool(name="wqk", bufs=2) as pWqk:
            for p in range(P8):
                wkt = pWqk.tile([128, DC, 128], BF16, tag="wk")
                wqt = pWqk.tile([128, DC, 128], BF16, tag="wq")
                nc.gpsimd.dma_start(wkt[:], wk_d[p])
                nc.gpsimd.dma_start(wqt[:], wq_d[p])
                for tb in range(4):
                    pk = psP.tile([128, 512], F32, tag="proj")
                    for dc in range(DC):
                        nc.tensor.matmul(pk[:], wkt[:, dc],
                                         X[:, dc, tb * 512:(tb + 1) * 512],
                                         start=(dc == 0), stop=(dc == DC - 1))
                    kt = KTlo if tb < 2 else KThi
                    nc.vector.tensor_copy(
                        kt[:, p, (tb % 2) * 512:(tb % 2 + 1) * 512], pk[:])
                for vb in range(2):
                    pq = psP.tile([128, 512], F32, tag="proj")
                    for dc in range(DC):
                        nc.tensor.matmul(pq[:], wqt[:, dc],
                                         XQ[:, dc, vb * 512:(vb + 1) * 512],
                                         start=(dc == 0), stop=(dc == DC - 1))
                    qt = QTA if vb == 0 else QTB
                    nc.vector.tensor_copy(qt[:, p], pq[:])

        # V projection
        with tc.tile_pool(name="pwv", bufs=1) as pWV:
            wvt = pWV.tile([128, DC, 1024], BF16, tag="wv")
            nc.sync.dma_start(wvt[:], wv_d[:])
            for tt in range(16):
                vt = Vlo if tt < 8 else Vhi
                for vh in range(2):
                    pv = psP.tile([128, 512], F32, tag="proj")
                    for dc in range(DC):
                        nc.tensor.matmul(
                            pv[:], X[:, dc, tt * 128:(tt + 1) * 128],
                            wvt[:, dc, vh * 512:(vh + 1) * 512],
                            start=(dc == 0), stop=(dc == DC - 1))
                    nc.vector.tensor_copy(
                        vt[:, tt % 8].rearrange(
                            "p (h c) -> p h c", c=65)[:, vh * 8:(vh + 1) * 8,
                                                      0:64],
                        pv[:].rearrange("p (h c) -> p h c", c=64))

        def attn_pair(p, nkc, qt, mask, mask_lo, ctxu, den):
            ctxA = psC.tile([65, 512], F32, tag="ctx")
            ctxB = psC.tile([65, 512], F32, tag="ctx")
            for kc in range(nkc):
                sA = psS.tile([128, 512], F32, tag="sc")
                sB = psS.tile([128, 512], F32, tag="sc")
                nc.tensor.matmul(sA[:], kt_sl(kc, slice(0, 64), p),
                                 qt[0:64, p], start=True, stop=True)
                nc.tensor.matmul(sB[:], kt_sl(kc, slice(64, 128), p),
                                 qt[64:128, p], start=True, stop=True)
                eA = pE.tile([128, 512], BF16, tag="E")
                eB = pE.tile([128, 512], BF16, tag="E")
                nc.scalar.activation(eA[:], sA[:], AF.Exp)
                nc.scalar.activation(eB[:], sB[:], AF.Exp)
                if (mask_lo and kc < 8) or (not mask_lo and kc >= 8):
                    nc.vector.tensor_mul(eA[:], eA[:], mask[:, kc % 8])
                    nc.vector.tensor_mul(eB[:], eB[:], mask[:, kc % 8])
                st, sp = (kc == 0), (kc == nkc - 1)
                nc.tensor.matmul(ctxA[:], v_sl(kc, 2 * p), eA[:],
                                 start=st, stop=sp)
                nc.tensor.matmul(ctxB[:], v_sl(kc, 2 * p + 1), eB[:],
                                 start=st, stop=sp)
            nc.vector.tensor_copy(ctxu[0:64, p], ctxA[0:64])
            nc.vector.tensor_copy(ctxu[64:128, p], ctxB[0:64])
            for h, cx in ((0, ctxA), (1, ctxB)):
                dt = pDT.tile([1, 512], F32, tag="dt")
                nc.vector.tensor_copy(dt[:], cx[64:65])
                nc.sync.dma_start(den[2 * p + h:2 * p + h + 1], dt[:])

        def attn_norm(ctxu, den):
            rec = pDEN.tile([16, 512], F32, tag="rec")
            nc.vector.reciprocal(rec[:], den[:])
            for p in range(P8):
                rtA = pDT.tile([1, 512], F32, tag="dt")
                nc.sync.dma_start(rtA[:], rec[2 * p:2 * p + 1])
                rtB = pDT.tile([1, 512], F32, tag="dt")
                nc.sync.dma_start(rtB[:], rec[2 * p + 1:2 * p + 2])
                rbt = pBC.tile([128, 512], F32, tag="rb")
                rbB = pBC.tile([64, 512], F32, tag="rbB")
                nc.gpsimd.partition_broadcast(rbt[0:64], rtA[:])
                nc.gpsimd.partition_broadcast(rbB[:], rtB[:])
                nc.sync.dma_start(rbt[64:128], rbB[:])
                nc.vector.tensor_mul(ctxu[:, p], ctxu[:, p], rbt[:])

        # ================= attention slot B (overlaps proj) ==========
        CTXU = pCTX.tile([128, P8, 512], BF16, tag="ctx", name="ctxB")
        DENB = pDEN.tile([16, 512], F32, tag="den", name="denB")
        for p in range(P8):
            attn_pair(p, NKC_B, QTB, MASKB, False, CTXU, DENB)
        attn_norm(CTXU, DENB)
        if dbg is not None:
            nc.gpsimd.dma_start(dbg["dbg_ctx"][:, :, 512:1024], CTXU[:])
        esP.close()  # X, stage, psP banks
        esB.close()  # KThi, QTB, Vhi, MASKB

        # ---- post-slot-B pools (right-side stack, live to the end) ----
        es2 = ExitStack()
        pWO = es2.enter_context(tc.tile_pool(name="wo", bufs=1, side="right"))
        pH1P = es2.enter_context(
            tc.tile_pool(name="h1p", bufs=1, side="right"))
        pH1 = es2.enter_context(tc.tile_pool(name="h1", bufs=1, side="right"))
        pWORK = es2.enter_context(
            tc.tile_pool(name="work", bufs=1, side="right"))
        pSTAT = es2.enter_context(
            tc.tile_pool(name="stat", bufs=1, side="right"))

        WO = pWO.tile([128, P8, DC, 128], BF16, tag="wo")
        nc.sync.dma_start(WO[:], wo_d[:])
        H1 = pH1.tile([128, DC, TLOC], BF16, tag="h1")

        def layer_norm(src, dst, gt, et, psPool, pstag):
            """LN over features of a [128, DC, 512] feature-major block."""
            sq = pWORK.tile([128, DC, 512], F32R, tag="work")
            nc.scalar.activation(sq[:], src, AF.Square)
            pmu = psPool.tile([1, 512], F32, tag=pstag)
            for dc in range(DC):
                nc.tensor.matmul(pmu[:], ones1[:], src[:, dc],
                                 start=(dc == 0), stop=(dc == DC - 1))
            mu = pSTAT.tile([1, 512], F32, tag="mu")
            nc.vector.tensor_scalar_mul(mu[:], pmu[:], 1.0 / D)
            psq = psPool.tile([1, 512], F32, tag=pstag)
            for dc in range(DC):
                nc.tensor.matmul(psq[:], ones1[:], sq[:, dc],
                                 start=(dc == 0), stop=(dc == DC - 1))
            ex2 = pSTAT.tile([1, 512], F32, tag="ex2")
            nc.vector.tensor_scalar_mul(ex2[:], psq[:], 1.0 / D)
            t = pSTAT.tile([1, 512], F32, tag="tmp")
            nc.vector.tensor_mul(t[:], mu[:], mu[:])
            nc.vector.tensor_sub(ex2[:], ex2[:], t[:])  # ex2 := var
            srt = pSTAT.tile([1, 512], F32, tag="tmp")
            nc.scalar.activation(srt[:], ex2[:], AF.Sqrt, bias=epst[:])
            rstd = pSTAT.tile([1, 512], F32, tag="rstd")
            nc.vector.reciprocal(rstd[:], srt[:])
            nmr = pSTAT.tile([1, 512], F32, tag="tmp")
            nc.vector.tensor_mul(nmr[:], mu[:], rstd[:])
            nc.vector.tensor_scalar_mul(nmr[:], nmr[:], -1.0)
            rstdb = pBC.tile([128, 512], F32, tag="bcln")
            nmrb = pBC.tile([128, 512], F32, tag="bcln")
            nc.gpsimd.partition_broadcast(rstdb[:], rstd[:])
            nc.gpsimd.partition_broadcast(nmrb[:], nmr[:])
            xh = pWORK.tile([128, DC, 512], F32R, tag="work")
            nc.vector.tensor_mul(xh[:], src,
                                 rstdb[:, None, :].to_broadcast((128, DC, 512)))
            nc.vector.tensor_add(xh[:], xh[:],
                                 nmrb[:, None, :].to_broadcast((128, DC, 512)))
            for dc in range(DC):
                nc.scalar.activation(dst[:, dc], xh[:, dc], AF.Identity,
                                     bias=et[:, dc:dc + 1],
                                     scale=gt[:, dc:dc + 1])

        def wo_ln1(ctxu, vsl, name):
            h1p = pH1P.tile([128, DC, 512], F32R, tag="h1p", name=name)
            for dout in range(DC):
                po = psW.tile([128, 512], F32, tag="wo")
                for p in range(P8):
                    nc.tensor.matmul(po[:], WO[:, p, dout], ctxu[:, p],
                                     start=(p == 0), stop=(p == P8 - 1))
                nc.vector.tensor_add(h1p[:, dout], po[:], XQ[:, dout, vsl])
            if dbg is not None:
                nc.gpsimd.dma_start(dbg["dbg_h1p"][:, :, vsl], h1p[:])
            layer_norm(h1p[:], H1[:, :, vsl], g1t, e1t, psW, "wo")
            if dbg is not None:
                nc.gpsimd.dma_start(dbg["dbg_h1"][:, :, vsl], H1[:, :, vsl])

        wo_ln1(CTXU, slice(512, 1024), "h1pB")

        # ================= attention slot A + FFN-B W1 ====================
        es3 = ExitStack()
        pFFT = es3.enter_context(
            tc.tile_pool(name="fft", bufs=1, side="right"))
        pW1s = es3.enter_context(
            tc.tile_pool(name="w1s", bufs=2, side="right"))
        FFT_B = pFFT.tile([128, FC, 512], BF16, tag="fft", name="fftB")

        def ffn_w1(fft, vsl, psPool):
            for ck in range(16):
                w1c = pW1s.tile([128, 2, DC, 128], BF16, tag="w1s")
                nc.sync.dma_start(
                    w1c[:], w1_d[ck * 2:(ck + 1) * 2].rearrange(
                        "f p dc n -> p f dc n"))
                for fi in range(2):
                    fc = ck * 2 + fi
                    pf = psPool.tile([128, 512], F32, tag="ff")
                    for dc in range(DC):
                        nc.tensor.matmul(pf[:], w1c[:, fi, dc],
                                         H1[:, dc, vsl],
                                         start=(dc == 0), stop=(dc == DC - 1))
                    nc.scalar.activation(fft[:, fc], pf[:], AF.Relu,
                                         bias=b1t[:, fc:fc + 1])

        with tc.tile_pool(name="psF", bufs=2, space="PSUM") as psF:
            CTXA = pCTX.tile([128, P8, 512], BF16, tag="ctx", name="ctxA")
            DENA = pDEN.tile([16, 512], F32, tag="den", name="denA")
            for p in range(P8):
                attn_pair(p, NKC_A, QTA, MASKA, True, CTXA, DENA)
            attn_norm(CTXA, DENA)
            if dbg is not None:
                nc.gpsimd.dma_start(dbg["dbg_ctx"][:, :, 0:512], CTXA[:])
            ffn_w1(FFT_B, slice(512, 1024), psF)
            wo_ln1(CTXA, slice(0, 512), "h1pA")

        esAttn.close()

        # ================= FFN W2 + second half =================
        with (
            tc.tile_pool(name="psG", bufs=4, space="PSUM") as psG,
            tc.tile_pool(name="psT", bufs=1, space="PSUM") as psT,
            tc.tile_pool(name="psF2", bufs=2, space="PSUM") as psF2,
            tc.tile_pool(name="pw2", bufs=2, side="right") as pW2,
            tc.tile_pool(name="po2", bufs=1, side="right") as pO2,
            tc.tile_pool(name="pout", bufs=2, side="right") as pOUT,
        ):
            def ffn_w2_ln2(fft, vsl, name):
                o2 = pO2.tile([128, DC, 512], F32R, tag="o2s", name=name)
                for g in range(2):
                    pos = [psG.tile([128, 512], F32, tag="o2",
                                    name=f"po_{name}_{g}_{i}")
                           for i in range(4)]
                    for fq in range(4):
                        w2c = pW2.tile([128, 8, 4, 128], BF16, tag="w2s")
                        nc.sync.dma_start(
                            w2c[:],
                            w2_d[fq * 8:(fq + 1) * 8, :,
                                 g * 4:(g + 1) * 4].rearrange(
                                     "f p d n -> p f d n"))
                        for fi in range(8):
                            for dd in range(4):
                                nc.tensor.matmul(
                                    pos[dd][:], w2c[:, fi, dd],
                                    fft[:, fq * 8 + fi],
                                    start=(fq == 0 and fi == 0),
                                    stop=(fq == 3 and fi == 7))
                    for dd in range(4):
                        dout = g * 4 + dd
                        nc.vector.scalar_tensor_tensor(
                            o2[:, dout], pos[dd][:], b2t[:, dout:dout + 1],
                            H1[:, dout, vsl], ALU.add, ALU.add)
                if dbg is not None:
                    nc.gpsimd.dma_start(dbg["dbg_o2"][:, :, vsl], o2[:])
                ot = pOUT.tile([128, DC, 512], F32, tag="out")
                layer_norm(o2[:], ot[:], g2t, e2t, psT, "stat")
                nc.sync.dma_start(
                    outT_d.rearrange("(dc p) t -> p dc t", p=128)[:, :, vsl],
                    ot[:])

            ffn_w2_ln2(FFT_B, slice(512, 1024), "o2B")
            FFT_A = pFFT.tile([128, FC, 512], BF16, tag="fft", name="fftA")
            ffn_w1(FFT_A, slice(0, 512), psF2)
            ffn_w2_ln2(FFT_A, slice(0, 512), "o2A")

        es3.close()
        es2.close()


def _pack_inputs(x, Wq, Wk, Wv, Wo, ln1_g, ln1_b, W1, b1, W2, b2, ln2_g, ln2_b):
    """Build the 8 per-core input maps (host-side numpy)."""
    import ml_dtypes
    f = np.float32
    bf = ml_dtypes.bfloat16
    x = np.asarray(x, f)
    Wq = np.asarray(Wq, f) * np.float32(SCALE)
    Wk = np.asarray(Wk, f)
    Wv = np.asarray(Wv, f)
    Wo = np.asarray(Wo, f)
    W1 = np.asarray(W1, f)
    W2 = np.asarray(W2, f)

    def pack_pair(W):
        # [H, D, 64] -> [P8, 128, DC, 128] (pair p = heads 2p, 2p+1)
        out = np.empty((P8, 128, DC, 128), f)
        for p in range(P8):
            w = np.concatenate([W[2 * p], W[2 * p + 1]], axis=1)  # [D, 128]
            out[p] = w.reshape(DC, 128, 128).transpose(1, 0, 2)
        return np.ascontiguousarray(out).astype(bf)

    wq = pack_pair(Wq)
    wk = pack_pair(Wk)
    wv = np.ascontiguousarray(
        np.concatenate([Wv[h] for h in range(H)], axis=1)
        .reshape(DC, 128, 1024).transpose(1, 0, 2)).astype(bf)
    wo = np.ascontiguousarray(
        Wo.reshape(P8, 128, DC, 128).transpose(1, 0, 2, 3)).astype(bf)
    w1 = np.ascontiguousarray(
        W1.reshape(DC, 128, FC, 128).transpose(2, 1, 0, 3)).astype(bf)
    w2 = np.ascontiguousarray(W2.reshape(FC, 128, DC, 128)).astype(bf)
    b1s = np.ascontiguousarray(np.asarray(b1, f).reshape(FC, 128).T)
    b2s = np.ascontiguousarray(np.asarray(b2, f).reshape(DC, 128).T)
    g1s = np.ascontiguousarray(np.asarray(ln1_g, f).reshape(DC, 128).T)
    e1s = np.ascontiguousarray(np.asarray(ln1_b, f).reshape(DC, 128).T)
    g2s = np.ascontiguousarray(np.asarray(ln2_g, f).reshape(DC, 128).T)
    e2s = np.ascontiguousarray(np.asarray(ln2_b, f).reshape(DC, 128).T)

    # diag planes: D[i][k, q] = 1 if i*128 + k <= q  (within a 512 q-block)
    kk = np.arange(512)[:, None]
    qq = np.arange(512)[None, :]
    diag = (kk <= qq).astype(f).reshape(4, 128, 512).transpose(1, 0, 2)
    ones = np.ones((128, 4, 512), f)
    zeros = np.zeros((128, 4, 512), f)
    # plane layout [128, 16, 512]: 0..7 = slot A (kc 0..7), 8..15 = slot B
    # (kc 8..15).  j=0: A=[diag(b0), zero(b1)], B=[diag(b3), ones(b2)]
    #              j=1: A=[ones(b0), diag(b1)], B=[diag(b2), zero(b3)]
    mask_j = [
        np.ascontiguousarray(
            np.concatenate([diag, zeros, diag, ones], axis=1)).astype(bf),
        np.ascontiguousarray(
            np.concatenate([ones, diag, diag, zeros], axis=1)).astype(bf),
    ]

    # X column order (real 512-blocks): j=0: [b0,b1,b3,b2]; j=1: [b0,b1,b2,b3]
    xorder = [[0, 1, 3, 2], [0, 1, 2, 3]]
    owned = [(0, 3), (1, 2)]  # (slot A block, slot B block)

    in_maps = []
    for c in range(NCORES):
        b, j = c // 2, c % 2
        xb = x[b]  # [S, D]
        xT = np.ascontiguousarray(
            np.concatenate([xb[o * 512:(o + 1) * 512] for o in xorder[j]]).T)
        bA, bB = owned[j]
        xq = np.ascontiguousarray(
            np.concatenate([xb[bA * 512:(bA + 1) * 512],
                            xb[bB * 512:(bB + 1) * 512]]).T)
        in_maps.append({
            "xT": xT, "xq": xq, "wq": wq, "wk": wk, "wv": wv, "wo": wo,
            "w1": w1, "w2": w2, "mask": mask_j[j],
            "b1s": b1s, "b2s": b2s, "g1s": g1s, "e1s": e1s,
            "g2s": g2s, "e2s": e2s,
        })
    return in_maps


def get_compiled():
    global _COMPILED
    if _COMPILED is None:
        _COMPILED = _build()
    return _COMPILED


def kernel(x, Wq, Wk, Wv, Wo, ln1_g, ln1_b, W1, b1, W2, b2, ln2_g, ln2_b,
           _trace=False):
    nc = get_compiled()
    in_maps = _pack_inputs(x, Wq, Wk, Wv, Wo, ln1_g, ln1_b, W1, b1, W2, b2,
                           ln2_g, ln2_b)
    res = run_bass_kernel_spmd(nc, in_maps, core_ids=list(range(NCORES)),
                               trace=_trace)
    owned = [(0, 3), (1, 2)]
    out = np.zeros((B, S, D), np.float32)
    for c in range(NCORES):
        b, j = c // 2, c % 2
        o = res.results[c]["outT"]  # [D, TLOC]
        bA, bB = owned[j]
        out[b, bA * 512:(bA + 1) * 512, :] = o[:, 0:512].T
        out[b, bB * 512:(bB + 1) * 512, :] = o[:, 512:1024].T
    kernel.last_result = res
    return out
